# revision 18
# baseline (speedup 1.0000x reference)
"""Trainium2 Bass kernel for nn_Discriminator_MD (GAN discriminator with
minibatch discrimination), distributed over 8 NeuronCores.

Strategy:
  - Convolutions batch-sharded (8 images/core), BatchNorm statistics merged
    with an AllReduce, BN+LeakyReLU applied as one fused Prelu activation.
  - MD2 projection column-sharded (each core owns 32 of 256 spatial positions
    of the projected features); the full normalized x2 is AllGathered, and the
    huge T2 matrix is streamed tile-by-tile from HBM (hw-contraction loop).
  - MD3/MD4 projections contraction-sharded over spatial positions with an
    AllReduce of the projected h.
  - The pairwise exp(-L1) stage runs as, per pair of batch columns: one
    (broadcast - identity) matmul producing all differences, one fused
    abs+segmented-reduce on the vector engine, one Exp activation, and one
    mask-matmul accumulating both the upper-triangle sums and the cumulative
    d(j, j+1) terms straight into PSUM.
"""

import os
import sys

import numpy as np

for _p in ("/opt/trn_rl_repo",):
    if _p not in sys.path:
        sys.path.insert(0, _p)

import concourse.bacc as bacc
import concourse.bass as bass
import concourse.mybir as mybir
import concourse.tile as tile
from concourse.bass_utils import run_bass_kernel_spmd

NC = 8          # cores
B = 64          # batch
BS = B // NC    # batch per core (8)
HID = 20
EPS = 1e-5

F = mybir.dt.float32
FR = mybir.dt.float32r

# fp32 = exact convs (4 cyc/row on PE); fp32r = ~1e-4 rounded, full speed.
CONV_DT = F

AX = mybir.AxisListType
ALU = mybir.AluOpType
ACT = mybir.ActivationFunctionType

_CACHED = {}


# ----------------------------------------------------------------------------
# device program
# ----------------------------------------------------------------------------

def _bn_block(nc, tc, pool, raw_aps, nfree, name, n_chunks):
    """BatchNorm stats for channel chunks (each raw_ap [P, nfree]) ->
    AllReduce -> per-chunk (rstd, bias) tiles. Returns list of (rstd, bias)."""
    Ps = [rap.shape[0] for rap in raw_aps]
    offs = np.cumsum([0] + Ps).tolist()
    C = offs[-1]
    bn_in = nc.dram_tensor(f"bn{name}_in", [C, 3], F)
    bn_out = nc.dram_tensor(f"bn{name}_out", [C, 3], F, addr_space="Shared")

    for k, rap in enumerate(raw_aps):
        P = Ps[k]
        nf = nfree
        # bn_stats free dim <= 512
        inner = 512 if nf % 512 == 0 else nf
        assert nf % inner == 0
        groups = nf // inner
        st6 = pool.tile([P, groups, 6], F, tag=f"st6_{name}_{k}")
        rap3 = rap.rearrange("p (g i) -> p g i", i=inner)
        for g in range(groups):
            nc.vector.bn_stats(st6[:, g, :], rap3[:, g, :])
        st2 = pool.tile([P, 2], F, tag=f"st2_{name}_{k}")
        nc.vector.bn_aggr(st2[:], st6[:].rearrange("p g i -> p (g i)"))
        st3 = pool.tile([P, 3], F, tag=f"st3_{name}_{k}")
        nc.vector.tensor_copy(st3[:, 0:2], st2[:])
        nc.vector.tensor_tensor(st3[:, 2:3], st2[:, 0:1], st2[:, 0:1],
                                op=ALU.mult)
        nc.sync.dma_start(out=bn_in.ap()[offs[k]:offs[k + 1], :], in_=st3[:])

    nc.gpsimd.collective_compute(
        "AllReduce", ALU.add, replica_groups=[list(range(NC))],
        ins=[bn_in.ap().opt()], outs=[bn_out.ap().opt()])

    out = []
    for k in range(n_chunks):
        P = Ps[k]
        t = pool.tile([P, 3], F, tag=f"bnr_{name}_{k}")
        nc.sync.dma_start(out=t[:], in_=bn_out.ap()[offs[k]:offs[k + 1], :])
        t8 = pool.tile([P, 3], F, tag=f"bnr8_{name}_{k}")
        nc.scalar.mul(t8[:], t[:], 1.0 / NC)          # (mu, Evar, Emu2)
        var = pool.tile([P, 1], F, tag=f"var_{name}_{k}")
        nc.vector.tensor_tensor(var[:], t8[:, 1:2], t8[:, 2:3], op=ALU.add)
        mu2 = pool.tile([P, 1], F, tag=f"mu2_{name}_{k}")
        nc.vector.tensor_tensor(mu2[:], t8[:, 0:1], t8[:, 0:1], op=ALU.mult)
        nc.vector.tensor_tensor(var[:], var[:], mu2[:], op=ALU.subtract)
        eps = pool.tile([P, 1], F, tag=f"eps_{name}_{k}")
        nc.vector.memset(eps[:], EPS)
        std = pool.tile([P, 1], F, tag=f"std_{name}_{k}")
        nc.scalar.activation(std[:], var[:], ACT.Sqrt, bias=eps[:, 0:1])
        rstd = pool.tile([P, 1], F, tag=f"rstd_{name}_{k}")
        nc.vector.reciprocal(rstd[:], std[:])
        bb = pool.tile([P, 1], F, tag=f"bb_{name}_{k}")
        nc.vector.scalar_tensor_tensor(bb[:], t8[:, 0:1], -1.0, rstd[:],
                                       op0=ALU.mult, op1=ALU.mult)
        out.append((rstd, bb))
    return out


def _pairwise(nc, tc, pool, psum, mbank, wbank, h_sb, m_cnt, md_out_ap, tag):
    """Minibatch-discrimination pairwise stage for m_cnt spatial positions.

    h_sb: SBUF f32r [64, m_cnt*HID].  Writes md [64, m_cnt] to md_out_ap."""
    free = m_cnt * HID
    md_ps = psum.tile([64, m_cnt], F, tag=f"mdps_{tag}", name="md_ps", bufs=1)
    for it in range(32):
        dneg = psum.tile([128, free], F, tag=f"dneg_{tag}")
        nc.tensor.matmul(dneg[:], mbank[:, it, :], h_sb,
                         start=True, stop=True)
        s = pool.tile([128, m_cnt], F, tag=f"s_{tag}")
        nc.vector.tensor_reduce(
            s[:], dneg[:].rearrange("p (m h) -> p m h", h=HID),
            axis=AX.X, op=ALU.add, apply_absolute_value=True)
        d2 = pool.tile([128, m_cnt], FR, tag=f"d2_{tag}")
        nc.scalar.activation(d2[:], s[:], ACT.Exp, scale=-1.0)
        nc.tensor.matmul(md_ps[:], wbank[:, it, :], d2[:],
                         start=(it == 0), stop=(it == 31))
    md_sb = pool.tile([64, m_cnt], F, tag=f"mdsb_{tag}")
    nc.scalar.copy(md_sb[:], md_ps[:])
    nc.sync.dma_start(out=md_out_ap, in_=md_sb[:])


def build_nc():
    nc = bacc.Bacc("TRN2", target_bir_lowering=False, debug=False,
                   num_devices=NC)
    CD = CONV_DT

    # ---------------- inputs ----------------
    xcol_d = nc.dram_tensor("xcol", [75, BS * 1024], CD, kind="ExternalInput")
    w1_d = nc.dram_tensor("w1t", [75, 64], CD, kind="ExternalInput")
    w2_d = nc.dram_tensor("w2t", [64, 25, 128], CD, kind="ExternalInput")
    w3a_d = nc.dram_tensor("w3ta", [128, 25, 256], CD, kind="ExternalInput")
    w3b_d = nc.dram_tensor("w3tb", [25, 256], CD, kind="ExternalInput")
    w4a_d = nc.dram_tensor("w4ta", [128, 25, 512], CD, kind="ExternalInput")
    w4b_d = nc.dram_tensor("w4tb", [128, 25, 512], CD, kind="ExternalInput")
    w4c_d = nc.dram_tensor("w4tc", [25, 512], CD, kind="ExternalInput")
    woc_d = nc.dram_tensor("woutc", [128, 4, 16], F, kind="ExternalInput")
    wom_d = nc.dram_tensor("woutmd", [1, 16], F, kind="ExternalInput")
    t2_d = nc.dram_tensor("t2r", [2, 256, 128, 320], FR, kind="ExternalInput")
    t3_d = nc.dram_tensor("t3r", [8, 2, 128, 1280], FR, kind="ExternalInput")
    t4_d = nc.dram_tensor("t4r", [2, 4, 128, 320], FR, kind="ExternalInput")
    mb_d = nc.dram_tensor("mbank", [64, 32, 128], FR, kind="ExternalInput")
    wb_d = nc.dram_tensor("wbank", [128, 32, 64], FR, kind="ExternalInput")

    # ---------------- outputs ----------------
    o_out = nc.dram_tensor("o_out", [BS, 1], F, kind="ExternalOutput")
    o_x1 = nc.dram_tensor("o_x1", [64, BS, 1024], F, kind="ExternalOutput")
    o_x2 = nc.dram_tensor("o_x2", [128, BS, 256], F, kind="ExternalOutput")
    o_x3 = nc.dram_tensor("o_x3", [256, BS, 64], F, kind="ExternalOutput")
    o_x4 = nc.dram_tensor("o_x4", [512, BS, 16], F, kind="ExternalOutput")
    o_md2 = nc.dram_tensor("o_md2", [64, 256], F, kind="ExternalOutput")
    o_md3 = nc.dram_tensor("o_md3", [64, 64], F, kind="ExternalOutput")
    o_md4 = nc.dram_tensor("o_md4", [64, 16], F, kind="ExternalOutput")

    # ---------------- collective bounce buffers ----------------
    agx2_i = nc.dram_tensor("agx2_i", [128, BS * 256], F)
    agx2_o = nc.dram_tensor("agx2_o", [NC * 128, BS * 256], F, addr_space="Shared")
    agx3_i = nc.dram_tensor("agx3_i", [256, BS * 64], F)
    agx3_o = nc.dram_tensor("agx3_o", [NC * 256, BS * 64], F, addr_space="Shared")
    agx4_i = nc.dram_tensor("agx4_i", [512, BS * 16], F)
    agx4_o = nc.dram_tensor("agx4_o", [NC * 512, BS * 16], F, addr_space="Shared")
    agm2_i = nc.dram_tensor("agm2_i", [64, 32], F)
    agm2_o = nc.dram_tensor("agm2_o", [NC * 64, 32], F, addr_space="Shared")
    agm3_i = nc.dram_tensor("agm3_i", [64, 8], F)
    agm3_o = nc.dram_tensor("agm3_o", [NC * 64, 8], F, addr_space="Shared")
    agm4_i = nc.dram_tensor("agm4_i", [64, 2], F)
    agm4_o = nc.dram_tensor("agm4_o", [NC * 64, 2], F, addr_space="Shared")
    arh3_i = nc.dram_tensor("arh3_i", [64, 1280], F)
    arh3_o = nc.dram_tensor("arh3_o", [64, 1280], F, addr_space="Shared")
    arh4_i = nc.dram_tensor("arh4_i", [64, 320], F)
    arh4_o = nc.dram_tensor("arh4_o", [64, 320], F, addr_space="Shared")

    def allgather(in_t, out_t):
        nc.gpsimd.collective_compute(
            "AllGather", ALU.bypass, replica_groups=[list(range(NC))],
            ins=[in_t.ap().opt()], outs=[out_t.ap().opt()])

    def allreduce(in_t, out_t):
        nc.gpsimd.collective_compute(
            "AllReduce", ALU.add, replica_groups=[list(range(NC))],
            ins=[in_t.ap().opt()], outs=[out_t.ap().opt()])

    from contextlib import ExitStack
    with tile.TileContext(nc) as tc, ExitStack() as stk:
        pers = stk.enter_context(tc.tile_pool(name="pers", bufs=1))

        pid_s = nc.sync.partition_id()
        pid_g = nc.gpsimd.partition_id()

        # ================= conv1 =================
        stkA = stk.enter_context(ExitStack())
        pool = stkA.enter_context(tc.tile_pool(name="poolA", bufs=1))
        psum = stkA.enter_context(tc.tile_pool(name="psumA", bufs=2, space="PSUM"))
        xcol = pool.tile([75, BS * 1024], CD, tag="xcol")
        w1 = pool.tile([75, 64], CD, tag="w1")
        nc.sync.dma_start(out=xcol[:], in_=xcol_d.ap())
        nc.sync.dma_start(out=w1[:], in_=w1_d.ap())
        x1raw = pool.tile([64, BS * 1024], F, tag="x1raw")
        for ch in range(BS * 2):
            ps = psum.tile([64, 512], F, tag="c1ps")
            nc.tensor.matmul(ps[:], w1[:], xcol[:, 512 * ch:512 * (ch + 1)],
                             start=True, stop=True)
            nc.scalar.copy(x1raw[:, 512 * ch:512 * (ch + 1)], ps[:])
        (rstd1, bb1), = _bn_block(nc, tc, pool, [x1raw[:]], BS * 1024, "1", 1)
        x1pad = pool.tile([64, BS, 36, 36], CD, tag="x1pad")
        nc.vector.memset(x1pad[:], 0)
        nc.scalar.activation(
            x1pad[:, :, 2:34, 2:34],
            x1raw[:].rearrange("p (b h w) -> p b h w", h=32, w=32),
            ACT.Prelu, bias=bb1[:, 0:1], scale=rstd1[:, 0:1], alpha=0.2)
        for b in range(BS):
            nc.sync.dma_start(out=o_x1.ap()[:, b, :],
                              in_=x1pad[:, b, 2:34, 2:34])

        # ================= conv2 =================
        w2 = pool.tile([64, 25, 128], CD, tag="w2")
        nc.sync.dma_start(out=w2[:], in_=w2_d.ap())
        x2raw = pool.tile([128, 4, 512], F, tag="x2raw")
        for ch in range(4):  # 2 images per chunk
            ps = psum.tile([128, 512], F, tag="c2ps")
            for kh in range(5):
                for kw in range(5):
                    tap = kh * 5 + kw
                    rhs = x1pad[:, 2 * ch:2 * ch + 2,
                                kh:kh + 32:2, kw:kw + 32:2]
                    nc.tensor.matmul(ps[:], w2[:, tap, :], rhs,
                                     start=(tap == 0), stop=(tap == 24))
            nc.scalar.copy(x2raw[:, ch, :], ps[:])
        (rstd2, bb2), = _bn_block(nc, tc, pool, [x2raw[:].rearrange("p a b -> p (a b)")],
                                  2048, "2", 1)
        x2pad = pers.tile([128, BS, 20, 20], CD, tag="x2pad")
        nc.vector.memset(x2pad[:], 0)
        nc.scalar.activation(
            x2pad[:, :, 2:18, 2:18],
            x2raw[:].rearrange("p c (b h w) -> p (c b) h w", b=2, h=16, w=16),
            ACT.Prelu, bias=bb2[:, 0:1], scale=rstd2[:, 0:1], alpha=0.2)
        agx2_i3 = agx2_i.ap().rearrange("c (b hw) -> c b hw", b=BS)
        for b in range(BS):
            nc.sync.dma_start(out=o_x2.ap()[:, b, :],
                              in_=x2pad[:, b, 2:18, 2:18])
            nc.sync.dma_start(out=agx2_i3[:, b, :],
                              in_=x2pad[:, b, 2:18, 2:18])
        allgather(agx2_i, agx2_o)
        stkA.close()

        # ================= MD2 =================
        mwpool = stk.enter_context(tc.tile_pool(name="mwpool", bufs=1))
        mbank = mwpool.tile([64, 32, 128], FR)
        wbank = mwpool.tile([128, 32, 64], FR)
        nc.sync.dma_start(out=mbank[:], in_=mb_d.ap())
        nc.sync.dma_start(out=wbank[:], in_=wb_d.ap())

        stkB = stk.enter_context(ExitStack())
        pool = stkB.enter_context(tc.tile_pool(name="poolB", bufs=1))
        psum = stkB.enter_context(tc.tile_pool(name="psumB", bufs=2, space="PSUM"))
        x2full = pool.tile([128, 64, 256], FR, tag="x2full")
        nc.gpsimd.dma_start(
            out=x2full[:],
            in_=agx2_o.ap().rearrange(
                "(core c) (b hw) -> c core b hw", c=128, hw=256))
        h2sb = pool.tile([64, 2, 320], FR, tag="h2sb")
        for half in range(2):
            hps = psum.tile([64, 320], F, tag="h2ps", name="h2ps", bufs=1)
            for hw in range(256):
                t2t = pool.tile([128, 320], FR, tag="t2t", bufs=4, name="t2t")
                nc.sync.dma_start(out=t2t[:], in_=t2_d.ap()[half, hw])
                nc.tensor.matmul(hps[:], x2full[:, :, hw], t2t[:],
                                 start=(hw == 0), stop=(hw == 255))
            nc.scalar.copy(h2sb[:, half, :], hps[:])
            _pairwise(nc, tc, pool, psum, mbank, wbank,
                      h2sb[:, half, :], 16,
                      agm2_i.ap()[:, 16 * half:16 * (half + 1)], "md2")
        allgather(agm2_i, agm2_o)
        nc.sync.dma_start(
            out=o_md2.ap(),
            in_=agm2_o.ap().rearrange("(core b) m -> b core m", b=64))
        md2pad = pers.tile([1, BS, 20, 20], CD, tag="md2pad")
        nc.vector.memset(md2pad[:], 0)
        agm2_v = agm2_o.ap().rearrange("(core b) m -> b core m", b=64)
        for b in range(BS):
            nc.sync.dma_start(
                out=md2pad[:, b, 2:18, 2:18].rearrange(
                    "a (c hl) w -> a c hl w", c=8),
                in_=agm2_v[bass.ds(pid_s * BS + b, 1)].rearrange(
                    "b core (hl w) -> b core hl w", w=16))
        stkB.close()

        # ================= conv3 =================
        stkC = stk.enter_context(ExitStack())
        pool = stkC.enter_context(tc.tile_pool(name="poolC", bufs=1))
        psum = stkC.enter_context(tc.tile_pool(name="psumC", bufs=2, space="PSUM"))
        w3a = pool.tile([128, 25, 256], CD, tag="w3a")
        w3b = pool.tile([25, 256], CD, tag="w3b")
        nc.sync.dma_start(out=w3a[:], in_=w3a_d.ap())
        nc.sync.dma_start(out=w3b[:], in_=w3b_d.ap())
        mdc2 = pool.tile([25, BS * 64], CD, tag="mdc2")
        md2eo = pool.tile([1, BS, 20, 2, 10], CD, tag="md2eo")
        nc.vector.tensor_copy(
            md2eo[:],
            md2pad[:].rearrange("a b r (q p) -> a b r p q", p=2))
        for kh in range(5):
            for kw in range(5):
                p0, q0 = kw % 2, kw // 2
                for b in range(BS):
                    nc.sync.dma_start(
                        out=mdc2[kh * 5 + kw:kh * 5 + kw + 1,
                                 64 * b:64 * (b + 1)].rearrange(
                                     "p (h w) -> p h w", h=8),
                        in_=md2eo[0:1, b, kh:kh + 16:2, p0, q0:q0 + 8])
        x3raw = pool.tile([128, 2, 512], F, tag="x3raw")
        for co in range(2):
            ps = psum.tile([128, 512], F, tag="c3ps")
            for kh in range(5):
                for kw in range(5):
                    tap = kh * 5 + kw
                    rhs = x2pad[:, :, kh:kh + 16:2, kw:kw + 16:2]
                    nc.tensor.matmul(ps[:], w3a[:, tap, 128 * co:128 * (co + 1)],
                                     rhs, start=(tap == 0), stop=False)
            nc.tensor.matmul(ps[:], w3b[:, 128 * co:128 * (co + 1)], mdc2[:],
                             start=False, stop=True)
            nc.scalar.copy(x3raw[:, co, :], ps[:])
        bn3 = _bn_block(nc, tc, pool,
                        [x3raw[:, 0, :], x3raw[:, 1, :]], 512, "3", 2)
        x3pad = pers.tile([128, 2, BS, 12, 12], CD, tag="x3pad")
        nc.vector.memset(x3pad[:], 0)
        for co in range(2):
            rstd, bb = bn3[co]
            nc.scalar.activation(
                x3pad[:, co, :, 2:10, 2:10],
                x3raw[:, co, :].rearrange("p (b h w) -> p b h w", h=8, w=8),
                ACT.Prelu, bias=bb[:, 0:1], scale=rstd[:, 0:1], alpha=0.2)
            agx3_i3 = agx3_i.ap()[128 * co:128 * (co + 1)].rearrange(
                "c (b hw) -> c b hw", b=BS)
            for b in range(BS):
                nc.sync.dma_start(out=o_x3.ap()[128 * co:128 * (co + 1), b, :],
                                  in_=x3pad[:, co, b, 2:10, 2:10])
                nc.sync.dma_start(out=agx3_i3[:, b, :],
                                  in_=x3pad[:, co, b, 2:10, 2:10])
        allgather(agx3_i, agx3_o)

        stkC.close()

        # ================= MD3 =================
        stkD = stk.enter_context(ExitStack())
        pool = stkD.enter_context(tc.tile_pool(name="poolD", bufs=1))
        psum = stkD.enter_context(tc.tile_pool(name="psumD", bufs=2, space="PSUM"))
        x3loc = pool.tile([128, 2, 64, 8], FR, tag="x3loc")
        agx3_v = agx3_o.ap().rearrange(
            "(core c2 c) (b hw) -> c c2 core b hw", c=128, c2=2, hw=64)
        for c2 in range(2):
            for core in range(NC):
                nc.gpsimd.dma_start(
                    out=x3loc[:, c2, BS * core:BS * (core + 1), :],
                    in_=agx3_v[:, c2, core, :, bass.ds(pid_g * 8, 8)])
        h3sb = pool.tile([64, 1280], F, tag="h3sb")
        hch = [(0, 512), (512, 512), (1024, 256)]
        hps3 = []
        for i, (_, w) in enumerate(hch):
            h3ps_t = psum.tile([64, w], F, tag=f"h3ps{i}", name=f"h3ps{i}", bufs=1)
            hps3.append(h3ps_t)
        for hwl in range(8):
            t3t = pool.tile([128, 2, 1280], FR, tag="t3t", bufs=2, name="t3t")
            nc.sync.dma_start(out=t3t[:], in_=t3_d.ap()[hwl])
            for c2 in range(2):
                for i, (n0, w) in enumerate(hch):
                    nc.tensor.matmul(hps3[i][:], x3loc[:, c2, :, hwl],
                                     t3t[:, c2, n0:n0 + w],
                                     start=(hwl == 0 and c2 == 0),
                                     stop=(hwl == 7 and c2 == 1))
        for i, (n0, w) in enumerate(hch):
            nc.scalar.copy(h3sb[:, n0:n0 + w], hps3[i][:])
        nc.sync.dma_start(out=arh3_i.ap(), in_=h3sb[:])
        allreduce(arh3_i, arh3_o)
        h3loc = pool.tile([64, 160], FR, tag="h3loc")
        nc.gpsimd.dma_start(out=h3loc[:],
                            in_=arh3_o.ap()[:, bass.ds(pid_g * 160, 160)])
        _pairwise(nc, tc, pool, psum, mbank, wbank, h3loc[:], 8,
                  agm3_i.ap(), "md3")
        allgather(agm3_i, agm3_o)
        nc.sync.dma_start(
            out=o_md3.ap(),
            in_=agm3_o.ap().rearrange("(core b) m -> b core m", b=64))
        md3pad = pers.tile([1, BS, 12, 12], CD, tag="md3pad")
        nc.vector.memset(md3pad[:], 0)
        agm3_v = agm3_o.ap().rearrange("(core b) m -> b core m", b=64)
        for b in range(BS):
            nc.sync.dma_start(
                out=md3pad[:, b, 2:10, 2:10],
                in_=agm3_v[bass.ds(pid_s * BS + b, 1)])

        stkD.close()

        # ================= conv4 =================
        stkE = stk.enter_context(ExitStack())
        pool = stkE.enter_context(tc.tile_pool(name="poolE", bufs=1))
        psum = stkE.enter_context(tc.tile_pool(name="psumE", bufs=2, space="PSUM"))
        w4a = pool.tile([128, 25, 512], CD, tag="w4a")
        w4b = pool.tile([128, 25, 512], CD, tag="w4b")
        w4c = pool.tile([25, 512], CD, tag="w4c")
        nc.sync.dma_start(out=w4a[:], in_=w4a_d.ap())
        nc.sync.dma_start(out=w4b[:], in_=w4b_d.ap())
        nc.sync.dma_start(out=w4c[:], in_=w4c_d.ap())
        mdc3 = pool.tile([25, BS * 16], CD, tag="mdc3")
        md3eo = pool.tile([1, BS, 12, 2, 6], CD, tag="md3eo")
        nc.vector.tensor_copy(
            md3eo[:],
            md3pad[:].rearrange("a b r (q p) -> a b r p q", p=2))
        for kh in range(5):
            for kw in range(5):
                p0, q0 = kw % 2, kw // 2
                for b in range(BS):
                    nc.sync.dma_start(
                        out=mdc3[kh * 5 + kw:kh * 5 + kw + 1,
                                 16 * b:16 * (b + 1)].rearrange(
                                     "p (h w) -> p h w", h=4),
                        in_=md3eo[0:1, b, kh:kh + 8:2, p0, q0:q0 + 4])
        x4raw = pool.tile([128, 4, 128], F, tag="x4raw")
        for cq in range(4):
            ps = psum.tile([128, 128], F, tag="c4ps")
            for c2, w4x in ((0, w4a), (1, w4b)):
                for kh in range(5):
                    for kw in range(5):
                        tap = kh * 5 + kw
                        rhs = x3pad[:, c2, :, kh:kh + 8:2, kw:kw + 8:2]
                        nc.tensor.matmul(
                            ps[:], w4x[:, tap, 128 * cq:128 * (cq + 1)], rhs,
                            start=(c2 == 0 and tap == 0), stop=False)
            nc.tensor.matmul(ps[:], w4c[:, 128 * cq:128 * (cq + 1)], mdc3[:],
                             start=False, stop=True)
            nc.scalar.copy(x4raw[:, cq, :], ps[:])
        bn4 = _bn_block(nc, tc, pool, [x4raw[:, k, :] for k in range(4)],
                        128, "4", 4)
        x4n = pers.tile([128, 4, BS, 16], CD, tag="x4n")
        for cq in range(4):
            rstd, bb = bn4[cq]
            nc.scalar.activation(
                x4n[:, cq, :, :],
                x4raw[:, cq, :].rearrange("p (b hw) -> p b hw", hw=16),
                ACT.Prelu, bias=bb[:, 0:1], scale=rstd[:, 0:1], alpha=0.2)
            nc.sync.dma_start(out=o_x4.ap()[128 * cq:128 * (cq + 1)],
                              in_=x4n[:, cq, :, :])
            nc.sync.dma_start(out=agx4_i.ap()[128 * cq:128 * (cq + 1)],
                              in_=x4n[:, cq, :, :])
        allgather(agx4_i, agx4_o)

        stkE.close()

        # ================= MD4 =================
        pool = stk.enter_context(tc.tile_pool(name="poolF", bufs=1))
        psum = stk.enter_context(tc.tile_pool(name="psumF", bufs=2, space="PSUM"))
        x4loc = pool.tile([128, 4, 64, 2], FR, tag="x4loc")
        agx4_v = agx4_o.ap().rearrange(
            "(core c4 c) (b hw) -> c c4 core b hw", c=128, c4=4, hw=16)
        for c4 in range(4):
            for core in range(NC):
                nc.gpsimd.dma_start(
                    out=x4loc[:, c4, BS * core:BS * (core + 1), :],
                    in_=agx4_v[:, c4, core, :, bass.ds(pid_g * 2, 2)])
        hps4 = psum.tile([64, 320], F, tag="h4ps", bufs=1)
        for hwl in range(2):
            t4t = pool.tile([128, 4, 320], FR, tag="t4t", bufs=2, name="t4t")
            nc.sync.dma_start(out=t4t[:], in_=t4_d.ap()[hwl])
            for c4 in range(4):
                nc.tensor.matmul(hps4[:], x4loc[:, c4, :, hwl], t4t[:, c4, :],
                                 start=(hwl == 0 and c4 == 0),
                                 stop=(hwl == 1 and c4 == 3))
        h4sb = pool.tile([64, 320], F, tag="h4sb")
        nc.scalar.copy(h4sb[:], hps4[:])
        nc.sync.dma_start(out=arh4_i.ap(), in_=h4sb[:])
        allreduce(arh4_i, arh4_o)
        h4loc = pool.tile([64, 40], FR, tag="h4loc")
        nc.gpsimd.dma_start(out=h4loc[:],
                            in_=arh4_o.ap()[:, bass.ds(pid_g * 40, 40)])
        _pairwise(nc, tc, pool, psum, mbank, wbank, h4loc[:], 2,
                  agm4_i.ap(), "md4")
        allgather(agm4_i, agm4_o)
        nc.sync.dma_start(
            out=o_md4.ap(),
            in_=agm4_o.ap().rearrange("(core b) m -> b core m", b=64))
        md4loc = pool.tile([1, BS, 16], F, tag="md4loc")
        agm4_v = agm4_o.ap().rearrange("(core b) m -> b core m", b=64)
        for b in range(BS):
            nc.sync.dma_start(
                out=md4loc[:, b, :],
                in_=agm4_v[bass.ds(pid_s * BS + b, 1)])

        # ================= final conv + sigmoid =================
        woc = pool.tile([128, 4, 16], F, tag="woc")
        wom = pool.tile([1, 16], F, tag="wom")
        nc.sync.dma_start(out=woc[:], in_=woc_d.ap())
        nc.sync.dma_start(out=wom[:], in_=wom_d.ap())
        fps = psum.tile([BS, 1], F, tag="fps", bufs=1)
        # x4n is CONV_DT; final matmuls in fp32 need fp32 operands
        x4f = pool.tile([128, 4, BS, 16], F, tag="x4f")
        nc.vector.tensor_copy(x4f[:], x4n[:])
        first = True
        for cq in range(4):
            for hw in range(16):
                nc.tensor.matmul(fps[:], x4f[:, cq, :, hw],
                                 woc[:, cq, hw:hw + 1],
                                 start=first, stop=False)
                first = False
        for hw in range(16):
            nc.tensor.matmul(fps[:], md4loc[:, :, hw], wom[:, hw:hw + 1],
                             start=False, stop=(hw == 15))
        osb = pool.tile([BS, 1], F, tag="osb")
        nc.scalar.activation(osb[:], fps[:], ACT.Sigmoid)
        nc.sync.dma_start(out=o_out.ap(), in_=osb[:])

    nc.compile()
    return nc


# ----------------------------------------------------------------------------
# host side
# ----------------------------------------------------------------------------

def _host_prep(x, w1, w2, w3, w4, w_out, T2, T3, T4):
    f = np.float32
    xp = np.zeros((64, 3, 68, 68), f)
    xp[:, :, 2:66, 2:66] = x
    sb, sc, sh, sw = xp.strides
    icol = np.lib.stride_tricks.as_strided(
        xp, shape=(3, 5, 5, 64, 32, 32),
        strides=(sc, sh, sw, sb, 2 * sh, 2 * sw))

    w1t = np.ascontiguousarray(w1.transpose(1, 2, 3, 0).reshape(75, 64))
    w2t = np.ascontiguousarray(w2.transpose(1, 2, 3, 0).reshape(64, 25, 128))
    w3ta = np.ascontiguousarray(w3[:, :128].transpose(1, 2, 3, 0).reshape(128, 25, 256))
    w3tb = np.ascontiguousarray(w3[:, 128].transpose(1, 2, 0).reshape(25, 256))
    w4ta = np.ascontiguousarray(w4[:, :128].transpose(1, 2, 3, 0).reshape(128, 25, 512))
    w4tb = np.ascontiguousarray(w4[:, 128:256].transpose(1, 2, 3, 0).reshape(128, 25, 512))
    w4tc = np.ascontiguousarray(w4[:, 256].transpose(1, 2, 0).reshape(25, 512))
    woutc = np.ascontiguousarray(w_out[0, :512].reshape(4, 128, 16).transpose(1, 0, 2))
    woutmd = np.ascontiguousarray(w_out[0, 512].reshape(1, 16))

    # pairwise constant banks
    m_bank = np.zeros((32, 64, 128), f)
    w_bank = np.zeros((32, 128, 64), f)
    ii = np.arange(64)
    for it in range(32):
        j0, j1 = 2 * it, 2 * it + 1
        m_bank[it, j0, :64] += 1
        m_bank[it, ii, ii] -= 1
        m_bank[it, j1, 64:] += 1
        m_bank[it, ii, 64 + ii] -= 1
        w_bank[it, ii[j0 + 1:], ii[j0 + 1:]] = 1          # D[i, j0], i < j0
        w_bank[it, 64 + ii[j1 + 1:], ii[j1 + 1:]] = 1     # D[i, j1], i < j1
        if j0 >= 1:
            w_bank[it, j0 - 1, j0:] += 1                  # e[j0-1]
        w_bank[it, 64 + j0, j0 + 1:] += 1                 # e[j0]
    m_bank = np.ascontiguousarray(m_bank.transpose(1, 0, 2))
    w_bank = np.ascontiguousarray(w_bank.transpose(1, 0, 2))

    T2r = T2.reshape(128, 256, 5120)
    T3r = T3.reshape(256, 64, 1280)
    T4r = T4.reshape(512, 16, 320)

    in_maps = []
    for c in range(NC):
        xcol = np.ascontiguousarray(
            icol[:, :, :, BS * c:BS * (c + 1)]).reshape(75, BS * 1024)
        t2s = T2r[:, :, 640 * c:640 * (c + 1)].transpose(1, 0, 2)  # [256,128,640]
        t2r = np.ascontiguousarray(
            np.stack([t2s[:, :, :320], t2s[:, :, 320:]]))          # [2,256,128,320]
        t3r = np.ascontiguousarray(
            T3r[:, 8 * c:8 * (c + 1)].transpose(1, 0, 2)).reshape(8, 2, 128, 1280)
        t4r = np.ascontiguousarray(
            T4r[:, 2 * c:2 * (c + 1)].transpose(1, 0, 2)).reshape(2, 4, 128, 320)
        in_maps.append(dict(
            xcol=xcol, w1t=w1t, w2t=w2t, w3ta=w3ta, w3tb=w3tb,
            w4ta=w4ta, w4tb=w4tb, w4tc=w4tc, woutc=woutc, woutmd=woutmd,
            t2r=t2r, t3r=t3r, t4r=t4r, mbank=m_bank, wbank=w_bank))
    return in_maps


def _assemble(results):
    out = np.zeros((64,), np.float32)
    x1 = np.zeros((64, 64, 32, 32), np.float32)
    x2 = np.zeros((64, 129, 16, 16), np.float32)
    x3 = np.zeros((64, 257, 8, 8), np.float32)
    x4 = np.zeros((64, 513, 4, 4), np.float32)
    for c in range(NC):
        r = results[c]
        sl = slice(BS * c, BS * (c + 1))
        out[sl] = r["o_out"][:, 0]
        x1[sl] = r["o_x1"].reshape(64, BS, 32, 32).transpose(1, 0, 2, 3)
        x2[sl, :128] = r["o_x2"].reshape(128, BS, 16, 16).transpose(1, 0, 2, 3)
        x3[sl, :256] = r["o_x3"].reshape(256, BS, 8, 8).transpose(1, 0, 2, 3)
        x4[sl, :512] = r["o_x4"].reshape(512, BS, 4, 4).transpose(1, 0, 2, 3)
    r0 = results[0]
    x2[:, 128] = r0["o_md2"].reshape(64, 16, 16)
    x3[:, 256] = r0["o_md3"].reshape(64, 8, 8)
    x4[:, 512] = r0["o_md4"].reshape(64, 4, 4)
    return out, x1, x2, x3, x4


def kernel(x, w1, w2, w3, w4, w_out, T2, T3, T4):
    x, w1, w2, w3, w4, w_out, T2, T3, T4 = [
        np.asarray(a, np.float32)
        for a in (x, w1, w2, w3, w4, w_out, T2, T3, T4)]
    if "nc" not in _CACHED:
        _CACHED["nc"] = build_nc()
    nc = _CACHED["nc"]
    in_maps = _host_prep(x, w1, w2, w3, w4, w_out, T2, T3, T4)
    res = run_bass_kernel_spmd(nc, in_maps, core_ids=list(range(NC)))
    return _assemble(res.results)


if __name__ == "__main__":
    # quick self-test against the saved reference outputs
    _ri = np.load(os.path.join(os.path.dirname(__file__), "ref_inputs.npz"))
    inputs = {k: _ri[k] for k in _ri.files}
    got = kernel(**inputs)
    ref = np.load(os.path.join(os.path.dirname(__file__), "ref_outputs.npz"))
    for name, g in zip(["out", "x1", "x2", "x3", "x4"], got):
        r = ref[name]
        err = np.abs(g - r).max()
        scale = np.abs(r).max() + 1e-30
        print(f"{name}: absmax_err={err:.4g} rel={err/scale:.4g}")
